# revision 14
# baseline (speedup 1.0000x reference)
"""InfoNCE loss kernel for Trainium2, 8 NeuronCores.

Problem: feature/feature_aug [B=8, T=256, V=32, D=256] fp32.
  scores = einsum('ivqd,jvkd->ijvqk'); E = exp(scores)
  ap[i,k]    = sum_{v,q} E[i,i,v,q,k]
  total[i,q] = sum_{j,v,k} E[i,j,v,q,k]
  self[i,q]  = sum_{v,k} E[i,i,v,q,k]
  loss = sum_i mean_t( log(total-self) - log(ap) )

Sharding: over V (32 -> 4 per core). Each core holds its 4 v-slices of BOTH
feature and feature_aug (8 MB each, resident in SBUF) and computes all (i,j)
pairs for its v's => only 16 MB DMA per core (vs 72 MB for i-sharding).

Per (v, i, qchunk): one PSUM strip [128q, 2048 = 8j x 256k] filled by 8
float32r matmuls (4 j-pair groups of N=512, 2 d-chunks each), then ONE
ScalarE exp over the whole strip with fused row-sum (accum_out) -> total
partial. Diagonal (j==i) 256-col slice: DVE row-sum -> self partial, and a
ones-vector matmul over the exp'd SBUF copy -> column sums -> ap partial.
Host sums the 8 cores' partials (v-partition) and takes logs.
"""

import numpy as np

import concourse.bass as bass
import concourse.mybir as mybir
import concourse.tile as tile
from concourse.bass_utils import run_bass_kernel_spmd

B, T, V, D = 8, 256, 32, 256
NCORES = 8
VPC = V // NCORES          # 4 v per core
DC = D // 128              # 2 d chunks
QC = T // 128              # 2 q chunks
NSTRIP = VPC * B * QC      # 64 strips per core
F32 = mybir.dt.float32
F32R = mybir.dt.float32r


def build_program():
    nc = bass.Bass()
    fq = nc.dram_tensor("fq", [VPC, D, B, T], F32R, kind="ExternalInput")
    fa = nc.dram_tensor("fa", [VPC, D, B, T], F32R, kind="ExternalInput")
    stats_d = nc.dram_tensor("stats", [128, NSTRIP], F32, kind="ExternalOutput")
    selfs_d = nc.dram_tensor("selfs", [128, NSTRIP], F32, kind="ExternalOutput")
    ap_d = nc.dram_tensor("ap", [1, B * T], F32, kind="ExternalOutput")

    with tile.TileContext(nc) as tc:
        with (
            tc.tile_pool(name="weights", bufs=1) as wpool,
            tc.tile_pool(name="psum", bufs=1, space="PSUM") as ppool,
            tc.tile_pool(name="escratch", bufs=1) as epool,
            tc.tile_pool(name="accs", bufs=1) as apool,
        ):
            ones_f = apool.tile([128, 1], F32, name="ones_f")
            nc.vector.memset(ones_f, 1.0)
            ones = apool.tile([128, 1], F32R, name="ones")
            nc.vector.tensor_copy(ones, ones_f)
            apacc = apool.tile([1, B * T], F32, name="apacc")
            nc.vector.memset(apacc, 0.0)

            FQ, FA = {}, {}
            for v in range(VPC):
                for dc in range(DC):
                    for nm, src_t, dst in (("fq", fq, FQ), ("fa", fa, FA)):
                        t = wpool.tile(
                            [128, B, T], F32R, tag=f"{nm}{v}{dc}", name=f"{nm}{v}{dc}"
                        )
                        nc.sync.dma_start(
                            out=t, in_=src_t[v, dc * 128:(dc + 1) * 128, :, :]
                        )
                        # in-place touch on DVE: collapses the matmuls'
                        # DMA-queue deps onto the one DVE sem
                        nc.vector.tensor_copy(t, t)
                        dst[v, dc] = t

            stripbufs = [
                ppool.tile([128, 2048], F32, tag=f"strip{k}", name=f"strip{k}")
                for k in range(2)
            ]
            ebufs = [
                epool.tile([128, 2048], F32R, tag=f"E{k}", name=f"E{k}")
                for k in range(3)
            ]
            stats = apool.tile([128, NSTRIP], F32, name="stats_sb")
            selfs = apool.tile([128, NSTRIP], F32, name="selfs_sb")

            # software pipeline: diag ops of strip s are emitted after strip
            # s+1's matmuls so PE never stalls waiting on ACT of strip s
            pending = None

            def emit_diag(p):
                (i0, E0, strip0, s0) = p
                dg = E0[:, i0 * 256:(i0 + 1) * 256]
                nc.vector.reduce_sum(
                    selfs[:, s0:s0 + 1], dg.bitcast(F32), axis=mybir.AxisListType.X
                )
                nc.tensor.matmul(
                    strip0[0:1, 1792:2048], lhsT=ones, rhs=dg, start=True, stop=True
                )
                asl = apacc[0:1, i0 * 256:(i0 + 1) * 256]
                nc.vector.tensor_add(asl, asl, strip0[0:1, 1792:2048])

            s = 0
            for v in range(VPC):
                for i in range(B):
                    for qc in range(QC):
                        strip = stripbufs[s % 2]
                        q0 = qc * 128
                        for jg in range(4):
                            for dc in range(DC):
                                nc.tensor.matmul(
                                    strip[:, jg * 512:(jg + 1) * 512],
                                    lhsT=FQ[v, dc][:, i, q0:q0 + 128],
                                    rhs=FA[v, dc][:, jg * 2:(jg + 1) * 2, :],
                                    start=(dc == 0), stop=(dc == DC - 1),
                                )
                        E = ebufs[s % 3]
                        nc.scalar.activation(
                            E, strip, mybir.ActivationFunctionType.Exp,
                            accum_out=stats[:, s:s + 1],
                        )
                        if pending is not None:
                            emit_diag(pending)
                        pending = (i, E, strip, s)
                        s += 1
            emit_diag(pending)

            nc.sync.dma_start(out=stats_d[:, :], in_=stats)
            nc.sync.dma_start(out=selfs_d[:, :], in_=selfs)
            nc.sync.dma_start(out=ap_d[:, :], in_=apacc)
    return nc


def _split_multi_waits(nc):
    """trn2 compute/DMA instructions carry at most ONE sync-wait slot in the
    ISA word; this walrus errors on more. Hoist extras onto NoOps queued just
    ahead on the same engine (in-order queues make this equivalent)."""
    for bb in nc.main_func.blocks:
        out = []
        for inst in bb.instructions:
            si = inst.sync_info
            if si is not None and si.on_wait and len(si.on_wait) > 1:
                for k, w in enumerate(si.on_wait[:-1]):
                    nop = mybir.InstNoOp(name=f"{inst.name}-sw{k}")
                    nop.engine = inst.engine
                    nop.sync_info = mybir.SyncInfo(on_wait=[w], on_update=[])
                    out.append(nop)
                inst.sync_info = mybir.SyncInfo(
                    on_wait=[si.on_wait[-1]], on_update=list(si.on_update)
                )
            out.append(inst)
        if len(out) != len(bb.instructions):
            bb.instructions = out
    return nc


def shard_inputs(feature, feature_aug):
    # [B,T,V,D] -> [V,D,B,T] so each SBUF weight tile [128d, 8i*256t] DMAs
    # with 8 KB contiguous runs per partition
    F = np.ascontiguousarray(np.transpose(np.asarray(feature, np.float32), (2, 3, 0, 1)))
    FA = np.ascontiguousarray(np.transpose(np.asarray(feature_aug, np.float32), (2, 3, 0, 1)))
    return [
        {"fq": F[VPC * c:VPC * (c + 1)], "fa": FA[VPC * c:VPC * (c + 1)]}
        for c in range(NCORES)
    ]


def combine(results):
    totals = np.zeros((B, T), np.float64)
    selfs = np.zeros((B, T), np.float64)
    aps = np.zeros((B, T), np.float64)
    for r in results:
        st = r["stats"].astype(np.float64).reshape(128, VPC, B, QC)
        se = r["selfs"].astype(np.float64).reshape(128, VPC, B, QC)
        # total[i, qc*128+p] += sum_v stats[p, v, i, qc]
        totals += st.sum(axis=1).transpose(1, 2, 0).reshape(B, T)
        selfs += se.sum(axis=1).transpose(1, 2, 0).reshape(B, T)
        aps += r["ap"].astype(np.float64).reshape(B, T)
    an = totals - selfs
    loss = (np.log(an) - np.log(aps)).sum() / float(T)
    return np.float32(loss)


_CACHE = {}


def run(inputs, trace=False, **kw):
    if "nc" not in _CACHE:
        _CACHE["nc"] = _split_multi_waits(build_program())
    nc = _CACHE["nc"]
    in_maps = shard_inputs(inputs["feature"], inputs["feature_aug"])
    res = run_bass_kernel_spmd(nc, in_maps, list(range(NCORES)), trace=trace, **kw)
    return combine(res.results), res


def kernel(feature, feature_aug):
    loss, _ = run({"feature": feature, "feature_aug": feature_aug})
    return loss
